# revision 1
# baseline (speedup 1.0000x reference)
"""Bidirectional Mamba block — Bass/Tile program builder for one TRN2 core.

Per-core = one batch element, SPMD over 8 cores (data-parallel over batch).
Layout: channels on partitions, time on the free dim. fp16 matmuls with fp32
PSUM accumulation; the selective scan runs per (d_block of 128 channels,
n of 16 states) with time split into NH pieces to bound SBUF (the B/C
partition-broadcast tiles are resident per piece only).
"""
import sys
sys.path.insert(0, "/opt/trn_rl_repo")

from contextlib import ExitStack

import concourse.bacc as bacc
import concourse.tile as tile
import concourse.mybir as mybir

FP16 = mybir.dt.float16
FP32 = mybir.dt.float32
AF = mybir.ActivationFunctionType
OP = mybir.AluOpType

D_MODEL = 768
D_INNER = 1536
D_STATE = 16
D_CONV = 4
DT_RANK = 48
NB_M = D_MODEL // 128   # 6  dm blocks
NB_J = D_INNER // 128   # 12 j blocks (d_inner)
CH = 512                # psum chunk (free dim)


def build(L=2048, NH=2, dirs=("f", "b"), pool_bgen=3, pool_cmul=0, dma2=True):
    HL = L // NH
    assert HL % CH == 0
    NCH = HL // CH          # chunks per time-piece
    NCF = L // CH           # chunks per full L

    nc = bacc.Bacc("TRN2", target_bir_lowering=False, debug=False)

    # ---------------- DRAM I/O ----------------
    xT16 = nc.dram_tensor("xT16", [D_MODEL, L], FP16, kind="ExternalInput")
    ident16 = nc.dram_tensor("ident16", [128, 128], FP16, kind="ExternalInput")
    ones_row16 = nc.dram_tensor("ones_row16", [1, 128], FP16, kind="ExternalInput")
    ones_col16 = nc.dram_tensor("ones_col16", [128, 1], FP16, kind="ExternalInput")
    sel16 = nc.dram_tensor("sel16", [2 * D_STATE, 2 * D_STATE * 128], FP16, kind="ExternalInput")
    fusion_wT16 = nc.dram_tensor("fusion_wT16", [2 * D_MODEL, D_MODEL], FP16, kind="ExternalInput")
    fusion_b = nc.dram_tensor("fusion_b", [D_MODEL, 1], FP32, kind="ExternalInput")
    W = {}
    for p in dirs:
        W[p, "in_wT16"] = nc.dram_tensor(f"{p}_in_wT16", [D_MODEL, 2 * D_INNER], FP16, kind="ExternalInput")
        W[p, "conv_w"] = nc.dram_tensor(f"{p}_conv_w", [D_INNER, D_CONV], FP32, kind="ExternalInput")
        W[p, "conv_b"] = nc.dram_tensor(f"{p}_conv_b", [D_INNER, 1], FP32, kind="ExternalInput")
        W[p, "xproj_wT16"] = nc.dram_tensor(f"{p}_xproj_wT16", [D_INNER, DT_RANK + 2 * D_STATE], FP16, kind="ExternalInput")
        W[p, "dt_wT16"] = nc.dram_tensor(f"{p}_dt_wT16", [DT_RANK, D_INNER], FP16, kind="ExternalInput")
        W[p, "dt_b"] = nc.dram_tensor(f"{p}_dt_b", [D_INNER, 1], FP32, kind="ExternalInput")
        W[p, "A"] = nc.dram_tensor(f"{p}_A", [D_INNER, D_STATE], FP32, kind="ExternalInput")
        W[p, "D"] = nc.dram_tensor(f"{p}_D", [D_INNER, 1], FP32, kind="ExternalInput")
        W[p, "out_wT16"] = nc.dram_tensor(f"{p}_out_wT16", [D_INNER, D_MODEL], FP16, kind="ExternalInput")
        W[p, "ln_g"] = nc.dram_tensor(f"{p}_ln_g", [D_MODEL, 1], FP32, kind="ExternalInput")
        W[p, "ln_b"] = nc.dram_tensor(f"{p}_ln_b", [D_MODEL, 1], FP32, kind="ExternalInput")
    outT = nc.dram_tensor("outT", [D_MODEL, L], FP32, kind="ExternalOutput")

    scr = {}
    for p in dirs:
        for nm in ("z", "uc", "dl", "du", "yg"):
            scr[p, nm] = nc.dram_tensor(f"scr_{p}_{nm}", [NB_J, 128, L], FP16, kind="Internal")
        for m in ("cat",):
            scr[p, m] = nc.dram_tensor(f"scr_{p}_{m}", [NB_M, 128, L], FP16, kind="Internal")
    xh16_d = nc.dram_tensor("scr_xh16", [NB_M, 128, L], FP16, kind="Internal")
    x16_d = nc.dram_tensor("scr_x16", [NB_M, 128, L], FP16, kind="Internal")

    with tile.TileContext(nc) as tc, ExitStack() as top, \
         nc.allow_low_precision("fp16 pipeline by design; fp32 where it matters"):
        singles = top.enter_context(tc.tile_pool(name="singles", bufs=1))
        dma = nc.default_dma_engine
        dmas = nc.scalar if dma2 else nc.default_dma_engine

        def load_cols(dram, nb, tag):
            """(nb*128, 1) DRAM -> (128, nb) SBUF tile; column j = block j."""
            t = singles.tile([128, nb], FP32, tag=tag)
            for j in range(nb):
                dma.dma_start(t[:, j:j + 1], dram[j * 128:(j + 1) * 128, :])
            return t

        ident = singles.tile([128, 128], FP16, tag="ident", name="ident")
        dma.dma_start(ident[:], ident16[:])
        epsb = singles.tile([128, 1], FP32, tag="epsb", name="epsb")
        nc.vector.memset(epsb[:], 1e-5)
        onesr = singles.tile([1, 128], FP16, tag="onesr", name="onesr")
        dma.dma_start(onesr[:], ones_row16[:])
        onesc = singles.tile([128, 1], FP16, tag="onesc", name="onesc")
        dma.dma_start(onesc[:], ones_col16[:])
        sel = singles.tile([2 * D_STATE, 2 * D_STATE * 128], FP16, tag="sel", name="sel")
        dma.dma_start(sel[:], sel16[:])

        # ============ P0: LayerNorm stats + xhat ============
        with ExitStack() as ph:
            pool = ph.enter_context(tc.tile_pool(name="p0", bufs=2))
            big = ph.enter_context(tc.tile_pool(name="p0big", bufs=1))
            psp = ph.enter_context(tc.tile_pool(name="p0ps", bufs=2, space="PSUM"))
            xt = [big.tile([128, L], FP16, tag=f"xt{k}", name=f"xt{k}") for k in range(NB_M)]
            for k in range(NB_M):
                dma.dma_start(xt[k][:], xT16[k * 128:(k + 1) * 128, :])
            xsq = [big.tile([128, L], FP16, tag=f"xsq{k}", name=f"xsq{k}") for k in range(NB_M)]
            for k in range(NB_M):
                nc.scalar.activation(xsq[k][:], xt[k][:], AF.Square)
            mu_row = big.tile([1, L], FP16, tag="murow_sb", name="murow_sb")
            m2_row = big.tile([1, L], FP16, tag="m2row_sb", name="m2row_sb")
            for c in range(NCF):
                s = slice(c * CH, (c + 1) * CH)
                ps = psp.tile([1, CH], FP32, tag="murow", name="murow")
                for k in range(NB_M):
                    nc.tensor.matmul(ps[:], onesc[:], xt[k][:, s],
                                     start=(k == 0), stop=(k == NB_M - 1))
                nc.scalar.copy(mu_row[:, s], ps[:])
                ps2 = psp.tile([1, CH], FP32, tag="m2row", name="m2row")
                for k in range(NB_M):
                    nc.tensor.matmul(ps2[:], onesc[:], xsq[k][:, s],
                                     start=(k == 0), stop=(k == NB_M - 1))
                nc.scalar.copy(m2_row[:, s], ps2[:])
            mu_bc = big.tile([128, L], FP16, tag="mu_bc", name="mu_bc")
            m2_bc = big.tile([128, L], FP16, tag="m2_bc", name="m2_bc")
            for c in range(NCF):
                s = slice(c * CH, (c + 1) * CH)
                bc_ps = psp.tile([128, CH], FP32, tag="bcps", name="bcps")
                nc.tensor.matmul(bc_ps[:], onesr[:], mu_row[:, s])
                nc.scalar.copy(mu_bc[:, s], bc_ps[:])
                bc_ps2 = psp.tile([128, CH], FP32, tag="bcps", name="bcps")
                nc.tensor.matmul(bc_ps2[:], onesr[:], m2_row[:, s])
                nc.scalar.copy(m2_bc[:, s], bc_ps2[:])
            mean_bc = big.tile([128, L], FP16, tag="mean_bc", name="mean_bc")
            nc.vector.tensor_scalar(mean_bc[:], mu_bc[:], 1.0 / D_MODEL, None, OP.mult)
            msq = big.tile([128, L], FP32, tag="msq", name="msq")
            nc.scalar.square(msq[:], mean_bc[:])
            var = big.tile([128, L], FP32, tag="var", name="var")
            nc.vector.scalar_tensor_tensor(var[:], m2_bc[:], 1.0 / D_MODEL, msq[:],
                                           OP.mult, OP.subtract)
            lnv = big.tile([128, L], FP32, tag="lnv", name="lnv")
            nc.scalar.activation(lnv[:], var[:], AF.Ln, bias=epsb[:])
            rstd = big.tile([128, L], FP16, tag="rstd", name="rstd")
            nc.scalar.activation(rstd[:], lnv[:], AF.Exp, scale=-0.5)
            for k in range(NB_M):
                xm = pool.tile([128, L], FP16, tag="xm", name="xm")
                nc.vector.tensor_tensor(xm[:], xt[k][:], mean_bc[:], OP.subtract)
                xh = pool.tile([128, L], FP16, tag="xh", name="xh")
                nc.vector.tensor_tensor(xh[:], xm[:], rstd[:], OP.mult)
                dma.dma_start(xh16_d[k], xh[:])
                dma.dma_start(x16_d[k], xt[k][:])

        # ============ per-direction pipeline ============
        for p in dirs:
            rev = (p == "b")
            dbl = singles.tile([DT_RANK, L], FP16, tag=f"dbl_{p}", name=f"dbl_{p}")
            bc_rows = singles.tile([2 * D_STATE, L], FP16, tag=f"bcr_{p}", name=f"bcr_{p}")

            # --- P1: xln, in_proj, conv, xproj, dt ---
            with ExitStack() as ph:
                xlnp = ph.enter_context(tc.tile_pool(name=f"{p}xln", bufs=1))
                wp = ph.enter_context(tc.tile_pool(name=f"{p}w", bufs=2))
                tp = ph.enter_context(tc.tile_pool(name=f"{p}tmp", bufs=2))
                upadp = ph.enter_context(tc.tile_pool(name=f"{p}upad", bufs=1))
                ucp = ph.enter_context(tc.tile_pool(name=f"{p}uc", bufs=1))
                psA = ph.enter_context(tc.tile_pool(name=f"{p}psA", bufs=2, space="PSUM"))
                psB = ph.enter_context(tc.tile_pool(name=f"{p}psB", bufs=2, space="PSUM"))

                gcol = load_cols(W[p, "ln_g"], NB_M, f"g_{p}")
                bcol = load_cols(W[p, "ln_b"], NB_M, f"b_{p}")
                xln = [xlnp.tile([128, L], FP16, tag=f"xln{k}", name=f"xln{k}") for k in range(NB_M)]
                for k in range(NB_M):
                    xh = tp.tile([128, L], FP16, tag="xh_in", name="xh_in")
                    dma.dma_start(xh[:], xh16_d[k])
                    dst = xln[k][:, ::-1] if rev else xln[k][:]
                    nc.vector.tensor_scalar(dst, xh[:], gcol[:, k:k + 1],
                                            bcol[:, k:k + 1], OP.mult, op1=OP.add)

                upad = [upadp.tile([128, L + D_CONV - 1], FP16, tag=f"up{j}", name=f"up{j}")
                        for j in range(NB_J)]
                for j in range(NB_J):
                    nc.vector.memset(upad[j][:, 0:D_CONV - 1], 0.0)
                for j in range(2 * NB_J):  # 0..11 -> u, 12..23 -> z
                    lhs = [wp.tile([128, 128], FP16, tag=f"inw{k}", name=f"inw{k}") for k in range(NB_M)]
                    for k in range(NB_M):
                        dma.dma_start(lhs[k][:],
                                      W[p, "in_wT16"][k * 128:(k + 1) * 128,
                                                      j * 128:(j + 1) * 128])
                    for c in range(NCF):
                        s = slice(c * CH, (c + 1) * CH)
                        ps = psA.tile([128, CH], FP32, tag="inps", name="inps")
                        for k in range(NB_M):
                            nc.tensor.matmul(ps[:], lhs[k][:], xln[k][:, s],
                                             start=(k == 0), stop=(k == NB_M - 1))
                        if j < NB_J:
                            nc.scalar.copy(
                                upad[j][:, D_CONV - 1 + c * CH:D_CONV - 1 + (c + 1) * CH],
                                ps[:])
                        else:
                            zt = tp.tile([128, CH], FP16, tag="zt", name="zt")
                            nc.scalar.activation(zt[:], ps[:], AF.Silu)
                            dma.dma_start(scr[p, "z"][j - NB_J][:, s], zt[:])

                cw = load_cols(W[p, "conv_w"][:, 0:1], NB_J, f"cw0_{p}")
                cws = [cw]
                for k in range(1, D_CONV):
                    cws.append(load_cols(W[p, "conv_w"][:, k:k + 1], NB_J, f"cw{k}_{p}"))
                cb = load_cols(W[p, "conv_b"], NB_J, f"cb_{p}")
                uc = [ucp.tile([128, L], FP16, tag=f"uc{j}", name=f"uc{j}") for j in range(NB_J)]
                for j in range(NB_J):
                    dg = [tp.tile([128, 128], FP16, tag=f"diag{k}", name=f"diag{k}") for k in range(D_CONV)]
                    for k in range(D_CONV):
                        nc.vector.tensor_scalar(dg[k][:], ident[:],
                                                cws[k][:, j:j + 1], None, OP.mult)
                    for c in range(NCF):
                        ps = psB.tile([128, CH], FP32, tag="cvps", name="cvps")
                        for k in range(D_CONV):
                            nc.tensor.matmul(ps[:], dg[k][:],
                                             upad[j][:, k + c * CH: k + c * CH + CH],
                                             start=(k == 0), stop=(k == D_CONV - 1))
                        nc.scalar.activation(uc[j][:, c * CH:(c + 1) * CH], ps[:],
                                             AF.Silu, bias=cb[:, j:j + 1])

                xpw = [wp.tile([128, DT_RANK + 2 * D_STATE], FP16, tag=f"xpw{j}", name=f"xpw{j}")
                       for j in range(NB_J)]
                for j in range(NB_J):
                    dma.dma_start(xpw[j][:],
                                  W[p, "xproj_wT16"][j * 128:(j + 1) * 128, :])
                for c in range(NCF):
                    s = slice(c * CH, (c + 1) * CH)
                    ps = psA.tile([DT_RANK, CH], FP32, tag="xpps", name="xpps", bufs=1)
                    psb = psA.tile([2 * D_STATE, CH], FP32, tag="xppsb", name="xppsb", bufs=1)
                    for j in range(NB_J):
                        nc.tensor.matmul(ps[:], xpw[j][:, 0:DT_RANK], uc[j][:, s],
                                         start=(j == 0), stop=(j == NB_J - 1))
                        nc.tensor.matmul(psb[:], xpw[j][:, DT_RANK:], uc[j][:, s],
                                         start=(j == 0), stop=(j == NB_J - 1))
                    nc.scalar.copy(dbl[:, s], ps[:])
                    nc.scalar.copy(bc_rows[:, s], psb[:])
                dtw = wp.tile([DT_RANK, D_INNER], FP16, tag="dtw", name="dtw")
                dma.dma_start(dtw[:], W[p, "dt_wT16"][:])
                dtb = load_cols(W[p, "dt_b"], NB_J, f"dtb_{p}")
                for j in range(NB_J):
                    dl = tp.tile([128, L], FP16, tag="dl", name="dl")
                    for c in range(NCF):
                        s = slice(c * CH, (c + 1) * CH)
                        ps = psB.tile([128, CH], FP32, tag="dtps", name="dtps")
                        nc.tensor.matmul(ps[:], dtw[:, j * 128:(j + 1) * 128],
                                         dbl[0:DT_RANK, s], start=True, stop=True)
                        # softplus(x + b) = ln(exp(x + b) + 1)
                        et = tp.tile([128, CH], FP32, tag="spexp", name="spexp")
                        nc.scalar.activation(et[:], ps[:], AF.Exp,
                                             bias=dtb[:, j:j + 1])
                        nc.scalar.activation(dl[:, s], et[:], AF.Ln, bias=1.0)
                    du = tp.tile([128, L], FP16, tag="du", name="du")
                    nc.vector.tensor_tensor(du[:], dl[:], uc[j][:], OP.mult)
                    dma.dma_start(scr[p, "dl"][j], dl[:])
                    dma.dma_start(scr[p, "du"][j], du[:])
                    dma.dma_start(scr[p, "uc"][j], uc[j][:])

            # --- P2/P3: scan (NH time pieces) + gating ---
            Acols = load_cols(W[p, "A"][:, 0:1], NB_J, f"A0_{p}")
            Acol = [Acols]
            for n in range(1, D_STATE):
                Acol.append(load_cols(W[p, "A"][:, n:n + 1], NB_J, f"A{n}_{p}"))
            Dcol = load_cols(W[p, "D"], NB_J, f"D_{p}")
            hlast = singles.tile([128, NB_J * D_STATE], FP32, tag=f"hl_{p}", name=f"hl_{p}")
            for h in range(NH):
                hs = slice(h * HL, (h + 1) * HL)
                with ExitStack() as ph:
                    bcp = ph.enter_context(tc.tile_pool(name=f"{p}bc{h}", bufs=1))
                    stp = ph.enter_context(tc.tile_pool(name=f"{p}st{h}", bufs=2))
                    wk = ph.enter_context(tc.tile_pool(name=f"{p}wk{h}", bufs=2))
                    psp = ph.enter_context(tc.tile_pool(name=f"{p}sps{h}", bufs=2, space="PSUM"))
                    ypsp = ph.enter_context(tc.tile_pool(name=f"{p}yps{h}", bufs=2, space="PSUM"))
                    Bbc = [bcp.tile([128, HL], FP16, tag=f"Bbc{n}", name=f"Bbc{n}") for n in range(D_STATE)]
                    Cbc = [bcp.tile([128, HL], FP16, tag=f"Cbc{n}", name=f"Cbc{n}") for n in range(D_STATE)]
                    for n in range(D_STATE):
                        for c in range(NCH):
                            s = slice(c * CH, (c + 1) * CH)
                            sg = slice(h * HL + c * CH, h * HL + (c + 1) * CH)
                            ps = psp.tile([128, CH], FP32, tag="bcps", name="bcps")
                            nc.tensor.matmul(ps[:], sel[:, n * 128:(n + 1) * 128],
                                             bc_rows[:, sg])
                            nc.scalar.copy(Bbc[n][:, s], ps[:])
                            ps2 = psp.tile([128, CH], FP32, tag="bcps", name="bcps")
                            nc.tensor.matmul(ps2[:], sel[:, (D_STATE + n) * 128:
                                                         (D_STATE + n + 1) * 128],
                                             bc_rows[:, sg])
                            nc.scalar.copy(Cbc[n][:, s], ps2[:])
                    for j in range(NB_J):
                        dlt = stp.tile([128, HL], FP16, tag="dlt", name="dlt")
                        dmas.dma_start(dlt[:], scr[p, "dl"][j][:, hs])
                        dut = stp.tile([128, HL], FP16, tag="dut", name="dut")
                        dmas.dma_start(dut[:], scr[p, "du"][j][:, hs])
                        yps = ypsp.tile([128, HL], FP32, tag="yps", name="yps")
                        for n in range(D_STATE):
                            at = wk.tile([128, HL], FP16, tag="at", name="at")
                            nc.scalar.activation(at[:], dlt[:], AF.Exp,
                                                 scale=Acol[n][:, j:j + 1])
                            bt = wk.tile([128, HL], FP16, tag="bt", name="bt")
                            beng = nc.gpsimd if (n % 4) < pool_bgen else nc.vector
                            beng.tensor_tensor(bt[:], dut[:], Bbc[n][:], OP.mult)
                            ht = wk.tile([128, HL], FP16, tag="ht", name="ht")
                            init = 0.0 if h == 0 else hlast[:, j * D_STATE + n:
                                                           j * D_STATE + n + 1]
                            nc.vector.tensor_tensor_scan(ht[:], at[:], bt[:], init,
                                                         OP.mult, OP.add)
                            if h < NH - 1:
                                nc.vector.tensor_copy(
                                    hlast[:, j * D_STATE + n:j * D_STATE + n + 1],
                                    ht[:, HL - 1:HL])
                            pt = wk.tile([128, HL], FP16, tag="pt", name="pt")
                            peng = nc.gpsimd if (n % 4) < pool_cmul else nc.vector
                            peng.tensor_tensor(pt[:], ht[:], Cbc[n][:], OP.mult)
                            for c in range(NCH):
                                s = slice(c * CH, (c + 1) * CH)
                                nc.tensor.matmul(yps[:, s], ident[:], pt[:, s],
                                                 start=(n == 0), stop=(n == D_STATE - 1))
                        uct = stp.tile([128, HL], FP16, tag="uct", name="uct")
                        dmas.dma_start(uct[:], scr[p, "uc"][j][:, hs])
                        szt = stp.tile([128, HL], FP16, tag="szt", name="szt")
                        dmas.dma_start(szt[:], scr[p, "z"][j][:, hs])
                        yd = wk.tile([128, HL], FP16, tag="yd", name="yd")
                        nc.vector.scalar_tensor_tensor(yd[:], uct[:], Dcol[:, j:j + 1],
                                                       yps[:], OP.mult, OP.add)
                        yg = wk.tile([128, HL], FP16, tag="yg", name="yg")
                        nc.vector.tensor_tensor(yg[:], yd[:], szt[:], OP.mult)
                        dmas.dma_start(scr[p, "yg"][j][:, hs], yg[:])

            # --- P4: out_proj + residual -> cat (DRAM) ---
            with ExitStack() as ph:
                opp = ph.enter_context(tc.tile_pool(name=f"{p}op", bufs=3))
                owp = ph.enter_context(tc.tile_pool(name=f"{p}ow", bufs=1))
                psp = ph.enter_context(tc.tile_pool(name=f"{p}ops", bufs=1, space="PSUM"))
                ow = [[None] * NB_M for _ in range(NB_J)]
                for j in range(NB_J):
                    for m in range(NB_M):
                        t = owp.tile([128, 128], FP16, tag=f"ow{j}_{m}", name=f"ow{j}_{m}")
                        dma.dma_start(t[:], W[p, "out_wT16"][j * 128:(j + 1) * 128,
                                                             m * 128:(m + 1) * 128])
                        ow[j][m] = t
                for c in range(NCF):
                    s = slice(c * CH, (c + 1) * CH)
                    pss = [psp.tile([128, CH], FP32, tag=f"ops{m}", name=f"ops{m}") for m in range(NB_M)]
                    for j in range(NB_J):
                        ygc = opp.tile([128, CH], FP16, tag="ygc", name="ygc")
                        dmas.dma_start(ygc[:], scr[p, "yg"][j][:, s])
                        for m in range(NB_M):
                            nc.tensor.matmul(pss[m][:], ow[j][m][:], ygc[:],
                                             start=(j == 0), stop=(j == NB_J - 1))
                    for m in range(NB_M):
                        x16t = opp.tile([128, CH], FP16, tag="x16t", name="x16t")
                        ct = opp.tile([128, CH], FP16, tag="ct", name="ct")
                        if rev:
                            cr = NCF - 1 - c
                            sr = slice(cr * CH, (cr + 1) * CH)
                            dmas.dma_start(x16t[:], x16_d[m][:, sr])
                            nc.vector.tensor_tensor(ct[:], pss[m][:, ::-1], x16t[:], OP.add)
                            dma.dma_start(scr[p, "cat"][m][:, sr], ct[:])
                        else:
                            dmas.dma_start(x16t[:], x16_d[m][:, s])
                            nc.vector.tensor_tensor(ct[:], pss[m][:], x16t[:], OP.add)
                            dma.dma_start(scr[p, "cat"][m][:, s], ct[:])

        # ============ P5: fusion ============
        with ExitStack() as ph:
            fwp = ph.enter_context(tc.tile_pool(name="fw", bufs=1))
            fop = ph.enter_context(tc.tile_pool(name="fo", bufs=3))
            psp = ph.enter_context(tc.tile_pool(name="fps", bufs=1, space="PSUM"))
            fb = load_cols(fusion_b, NB_M, "fb")
            cat_d = [scr[dirs[0], "cat"][m] for m in range(NB_M)] + \
                    [scr[dirs[-1], "cat"][m] for m in range(NB_M)]
            fw = [[None] * NB_M for _ in range(2 * NB_M)]
            for cbk in range(2 * NB_M):
                for m in range(NB_M):
                    t = fwp.tile([128, 128], FP16, tag=f"fw{cbk}_{m}", name=f"fw{cbk}_{m}")
                    dma.dma_start(t[:], fusion_wT16[cbk * 128:(cbk + 1) * 128,
                                                    m * 128:(m + 1) * 128])
                    fw[cbk][m] = t
            for c in range(NCF):
                s = slice(c * CH, (c + 1) * CH)
                pss = [psp.tile([128, CH], FP32, tag=f"fps{m}", name=f"fps{m}") for m in range(NB_M)]
                for cbk in range(2 * NB_M):
                    catc = fop.tile([128, CH], FP16, tag="catc", name="catc")
                    dmas.dma_start(catc[:], cat_d[cbk][:, s])
                    for m in range(NB_M):
                        nc.tensor.matmul(pss[m][:], fw[cbk][m][:], catc[:],
                                         start=(cbk == 0), stop=(cbk == 2 * NB_M - 1))
                for m in range(NB_M):
                    ot = fop.tile([128, CH], FP32, tag="ot", name="ot")
                    nc.scalar.activation(ot[:], pss[m][:], AF.Identity,
                                         bias=fb[:, m:m + 1])
                    dma.dma_start(outT[m * 128:(m + 1) * 128, s], ot[:])

    nc.compile()
    return nc


def make_in_map(inputs_np, core, L=2048, dirs=("f", "b")):
    """Build the per-core input map from full numpy inputs (reference layout)."""
    import numpy as np
    x = inputs_np["x"]  # (B, L, D_MODEL)
    m = {
        "xT16": np.ascontiguousarray(x[core].T).astype(np.float16),
        "ident16": np.eye(128, dtype=np.float16),
        "ones_row16": np.ones((1, 128), np.float16),
        "ones_col16": np.ones((128, 1), np.float16),
        "sel16": np.kron(np.eye(2 * D_STATE, dtype=np.float16),
                         np.ones((1, 128), np.float16)).reshape(2 * D_STATE, -1),
        "fusion_wT16": np.ascontiguousarray(inputs_np["fusion_w"].T).astype(np.float16),
        "fusion_b": inputs_np["fusion_b"].reshape(D_MODEL, 1).astype(np.float32),
    }
    for p in dirs:
        m[f"{p}_in_wT16"] = np.ascontiguousarray(inputs_np[f"{p}_in_w"].T).astype(np.float16)
        m[f"{p}_conv_w"] = inputs_np[f"{p}_conv_w"].astype(np.float32)
        m[f"{p}_conv_b"] = inputs_np[f"{p}_conv_b"].reshape(D_INNER, 1).astype(np.float32)
        m[f"{p}_xproj_wT16"] = np.ascontiguousarray(inputs_np[f"{p}_xproj_w"].T).astype(np.float16)
        m[f"{p}_dt_wT16"] = np.ascontiguousarray(inputs_np[f"{p}_dt_w"].T).astype(np.float16)
        m[f"{p}_dt_b"] = inputs_np[f"{p}_dt_b"].reshape(D_INNER, 1).astype(np.float32)
        m[f"{p}_A"] = (-np.exp(inputs_np[f"{p}_A_log"])).astype(np.float32)
        m[f"{p}_D"] = inputs_np[f"{p}_D"].reshape(D_INNER, 1).astype(np.float32)
        m[f"{p}_out_wT16"] = np.ascontiguousarray(inputs_np[f"{p}_out_w"].T).astype(np.float16)
        m[f"{p}_ln_g"] = inputs_np[f"{p}_ln_g"].reshape(D_MODEL, 1).astype(np.float32)
        m[f"{p}_ln_b"] = inputs_np[f"{p}_ln_b"].reshape(D_MODEL, 1).astype(np.float32)
    return m


# ============================================================================
# SPMD runner: full inputs in, full output out (8 cores, batch-parallel)
# ============================================================================
_NC_CACHE = None


def _get_nc():
    global _NC_CACHE
    if _NC_CACHE is None:
        _NC_CACHE = build(L=2048, NH=2, pool_bgen=4, pool_cmul=0, dma2=False)
    return _NC_CACHE


def kernel(**inputs):
    import numpy as np
    inputs = {k: np.asarray(v) for k, v in inputs.items()}
    nc = _get_nc()
    B = inputs["x"].shape[0]
    assert B == 8
    in_maps = [make_in_map(inputs, c) for c in range(B)]
    from concourse.bass_utils import run_bass_kernel_spmd
    res = run_bass_kernel_spmd(nc, in_maps, core_ids=list(range(B)))
    out = np.stack([np.ascontiguousarray(res.results[c]["outT"].T) for c in range(B)], 0)
    return out.astype(np.float32)

